# revision 11
# baseline (speedup 1.0000x reference)
"""FP8 quantized matmul kernel for Trainium2 (8 NeuronCores, SPMD).

Computes: out = fp8_quant(input) @ fp8_quant(other), bf16 output.
  input: [16384, 2048] fp32, other: [2048, 2048] fp32.

Sharding: data-parallel over M. Each core processes 2048 rows of `input`
and a full replica of `other`; no cross-core communication. During
host-side sharding both operands are packed K-major into 512-wide
panel-of-column blocks ([128ki, panel, ko, 512] fp32), so every device
load is per-partition contiguous (8 KB lines, peak HBM efficiency) and
no on-device transposes are needed.

Per-core pipeline (all on device):
  1. A panels (input^T columns) and B panels (other columns) stream in
     as [128, 4ko, 512] fp32 chunks on the sync-engine HWDGE queue,
     interleaved A/B so the first output tile's operands land first.
     A chunks quantize fp32 -> fp8e4m3 on the Vector engine, B chunks on
     the Scalar engine (RNE saturating cast, exactly matching the
     reference quant for ~N(0,1) data where the +-448 clip never fires)
     into SBUF-resident qat / qb.
  2. FP8 DoubleRow matmuls (K paired 2x128) accumulate fp32 in PSUM.
     Output tiles are emitted in wavefront order over (m-group, n-panel)
     pairs so the Tensor engine starts as soon as the first k-chunks of
     (A g0, B p0) are quantized.
  3. PSUM evicts to bf16 on alternating Vector/Scalar engines and stores
     via the Scalar-engine HWDGE queue (separate from the load queue so
     store waits never block load issue), batched [128, 4, 512].
"""

import numpy as np

P = 128
M_LOC, K, N = 2048, 2048, 2048
N_CORES = 8
KO = K // P       # 16 k-blocks of 128
KP = KO // 2      # 8 DoubleRow k-pairs
FD = 512          # matmul free dim (one PSUM bank of fp32)
NT = N // FD      # 4 n panels
MG = M_LOC // FD  # 4 m groups (512 wide)
MI = FD // P      # 4 m slices per group
CKO = 4           # ko blocks per streamed chunk
KC = KO // CKO    # 4 k-chunks per panel/group

import os
XF_BUFS = int(os.environ.get('XF_BUFS', '3'))
WF_BUFS = int(os.environ.get('WF_BUFS', '3'))
OSB_BUFS = int(os.environ.get('OSB_BUFS', '4'))
PSUM_BUFS = int(os.environ.get('PSUM_BUFS', '8'))
OUT_ENG = os.environ.get('OUT_ENG', 'scalar')  # scalar | gpsimd | sync


def build(tc, xp, wp, out, iters=1, hw_loop=False):
    """Emit the per-core kernel IR. xp: [128, MG, KO, FD] f32 (the input
    shard, K-major panel-packed), wp: [128, NT, KO, FD] f32 (other,
    panel-packed), out: [M_LOC,N] bf16 (all DRAM APs). iters>1 repeats
    the whole computation (python-unrolled, or a hardware For_i loop when
    hw_loop=True) for marginal-time benchmarking."""
    import contextlib

    import concourse.mybir as mybir

    nc = tc.nc
    f32 = mybir.dt.float32
    bf16 = mybir.dt.bfloat16
    fp8 = mybir.dt.float8e4

    out_r = out.rearrange("(t p) n -> p t n", p=P)  # m row = t*128 + p

    with (
        tc.tile_pool(name="resident", bufs=1) as resident,
        tc.tile_pool(name="stage", bufs=4) as stage,
        tc.tile_pool(name="ostage", bufs=4) as ostage,
        tc.tile_pool(name="psum_mm", bufs=PSUM_BUFS, space="PSUM") as psum_mm,
    ):
        if hw_loop:
            loop_ctx = tc.For_i(0, iters, 1)
            reps = 1
        else:
            loop_ctx = contextlib.nullcontext()
            reps = iters

        with loop_ctx:
            for _ in range(reps):
                _emit_body(tc, xp, wp, out_r, resident, stage, ostage,
                           psum_mm, mybir, f32, bf16, fp8)


def _emit_body(tc, xp, wp, out_r, resident, stage, ostage, psum_mm,
               mybir, f32, bf16, fp8):
    nc = tc.nc

    # [ki, g, ko, m] = quant(input)^T at k = ko*128 + ki, m = g*512 + m
    qat = resident.tile([P, MG, KO, FD], fp8, tag="qat")
    # [ki, p, ko, n] = quant(other) at k = ko*128 + ki, n = p*512 + n
    qb = resident.tile([P, NT, KO, FD], fp8, tag="qb")

    QCAST = os.environ.get('QCAST', '')  # chars 'x'/'w': quantize via SWDGE DMA cast

    def load_quant_chunk(src, dst, col, ko0, nko, which):
        """DMA one [128, nko, FD] f32 chunk (ko blocks [ko0, ko0+nko) of
        panel `col`, per-partition contiguous in DRAM) and quantize it
        into the fp8 resident tile. Staged in CKO-sized slots so small
        head chunks and full chunks share one pool tag."""
        ks = slice(ko0, ko0 + nko)
        if which in QCAST:
            # gpsimd SWDGE casts f32 -> fp8 in flight: no stage buffer,
            # no quant engine work.
            nc.gpsimd.dma_start(dst[:, col, ks, :], src[:, col, ks, :])
            return
        f = stage.tile(
            [P, CKO, FD], f32, tag=f"{which}f", name=f"{which}f_{col}_{ko0}",
            bufs=(XF_BUFS if which == "x" else WF_BUFS),
        )
        nc.sync.dma_start(f[:, :nko, :], src[:, col, ks, :])
        eng = nc.vector.tensor_copy if which == "x" else nc.scalar.copy
        eng(dst[:, col, ks, :], f[:, :nko, :])

    def mm_tile(g, p):
        """All MMs for output tile (m-group g, n-panel p): 4 m-slices of
        [128, 512], each accumulating 8 DoubleRow k-pairs in PSUM."""
        osb = ostage.tile(
            [P, MI, FD], bf16, tag="osb", name=f"osb_{g}_{p}", bufs=OSB_BUFS
        )
        for mi in range(MI):
            ps = psum_mm.tile([P, FD], f32, tag="ps", name=f"ps_{g}_{p}_{mi}")
            for kp in range(KP):
                nc.tensor.matmul(
                    ps,
                    qat[:, g, 2 * kp : 2 * kp + 2, mi * P : (mi + 1) * P],
                    qb[:, p, 2 * kp : 2 * kp + 2, :],
                    start=(kp == 0),
                    stop=(kp == KP - 1),
                    perf_mode=mybir.MatmulPerfMode.DoubleRow,
                )
            if mi == MI - 1:
                nc.scalar.copy(osb[:, mi, :], ps)
            else:
                nc.vector.tensor_copy(osb[:, mi, :], ps)
        out_eng = {"scalar": nc.scalar, "gpsimd": nc.gpsimd, "sync": nc.sync}[OUT_ENG]
        out_eng.dma_start(
            out_r[:, g * MI : (g + 1) * MI, p * FD : (p + 1) * FD], osb
        )

    # k-chunk splits: small head chunks for the first pair so the tensor
    # engine starts as early as possible.
    HEAD = [(0, 2), (2, 2), (4, 4), (8, 4), (12, 4)]
    FINE = [(0, 2), (2, 2), (4, 2), (6, 2), (8, 2), (10, 2), (12, 2), (14, 2)]
    FULL = [(0, 4), (4, 4), (8, 4), (12, 4)]

    # B-ahead streaming: phase 1 round-robins (A0 | B0 | B1) so two B
    # panels land by the time A0's tiles are consumed; later phases keep
    # >=2 streams chunk-interleaved (single-stream issue loses ~20% DMA
    # bandwidth to stage-buffer stalls on the in-order sync queue).
    # Tiles are emitted the moment their last operand's load has been
    # issued, so the PE queue order matches data arrival. The
    # last-arriving panel (A3) gates only 4 tiles of PE work.
    SCHED = os.environ.get('SCHED', 'a')
    if SCHED == 'a':
        phases = [
            ([("x", 0, HEAD), ("w", 0, HEAD), ("w", 1, FULL)],
             [(0, 0), (0, 1)]),
            ([("w", 2, FULL), ("x", 1, FULL)],
             [(0, 2), (1, 0), (1, 1), (1, 2)]),
            ([("w", 3, FULL), ("x", 2, FULL)],
             [(0, 3), (1, 3), (2, 0), (2, 1), (2, 2), (2, 3)]),
            ([("x", 3, FULL)],
             [(3, 0), (3, 1), (3, 2), (3, 3)]),
        ]
    elif SCHED == 'b':  # 4-way phase 1
        phases = [
            ([("x", 0, HEAD), ("w", 0, HEAD), ("w", 1, FULL), ("w", 2, FULL)],
             [(0, 0), (0, 1), (0, 2)]),
            ([("w", 3, FULL), ("x", 1, FULL)],
             [(0, 3), (1, 0), (1, 1), (1, 2), (1, 3)]),
            ([("x", 2, FULL), ("x", 3, FULL)],
             [(2, 0), (2, 1), (2, 2), (2, 3), (3, 0), (3, 1), (3, 2), (3, 3)]),
        ]
    else:  # 'c': fine chunks for pair 0
        phases = [
            ([("x", 0, FINE), ("w", 0, FINE), ("w", 1, FULL)],
             [(0, 0), (0, 1)]),
            ([("w", 2, FULL), ("x", 1, FULL)],
             [(0, 2), (1, 0), (1, 1), (1, 2)]),
            ([("w", 3, FULL), ("x", 2, FULL)],
             [(0, 3), (1, 3), (2, 0), (2, 1), (2, 2), (2, 3)]),
            ([("x", 3, FULL)],
             [(3, 0), (3, 1), (3, 2), (3, 3)]),
        ]
    for loads, tiles in phases:
        nchunks = max(len(cl) for _, _, cl in loads)
        for i in range(nchunks):
            for which, panel, chunklist in loads:
                if i < len(chunklist):
                    k0, nko = chunklist[i]
                    load_quant_chunk(
                        xp if which == "x" else wp,
                        qat if which == "x" else qb,
                        panel, k0, nko, which,
                    )
        for g, p in tiles:
            mm_tile(g, p)


def build_program(iters=1):
    """Build and compile the single-core SPMD program."""
    import concourse.bacc as bacc
    import concourse.mybir as mybir
    import concourse.tile as tile

    nc = bacc.Bacc("TRN2", target_bir_lowering=False, debug=False)
    xp = nc.dram_tensor(
        "xp", [P, MG, KO, FD], mybir.dt.float32, kind="ExternalInput"
    ).ap()
    wp = nc.dram_tensor(
        "wp", [P, NT, KO, FD], mybir.dt.float32, kind="ExternalInput"
    ).ap()
    out = nc.dram_tensor(
        "out", [M_LOC, N], mybir.dt.bfloat16, kind="ExternalOutput"
    ).ap()
    with tile.TileContext(nc) as tc:
        build(tc, xp, wp, out, iters=iters)
    nc.compile()
    return nc


_PROGRAM_CACHE = {}


def _pack_panels(a_t_like):
    """[K, C] fp32 -> [128ki, C/512 panel, 16ko, 512] (k = ko*128 + ki)."""
    return np.ascontiguousarray(
        a_t_like.reshape(KO, P, -1, FD).transpose(1, 2, 0, 3)
    )


def make_in_maps(input, other):
    input = np.asarray(input, dtype=np.float32)
    other = np.asarray(other, dtype=np.float32)
    wp = _pack_panels(other)
    return [
        {
            "xp": _pack_panels(input[c * M_LOC : (c + 1) * M_LOC].T),
            "wp": wp,
        }
        for c in range(N_CORES)
    ]


def kernel(input, other):
    from concourse.bass_utils import run_bass_kernel_spmd

    if "nc" not in _PROGRAM_CACHE:
        _PROGRAM_CACHE["nc"] = build_program()
    nc = _PROGRAM_CACHE["nc"]

    in_maps = make_in_maps(input, other)
    res = run_bass_kernel_spmd(nc, in_maps, list(range(N_CORES)))
    return np.concatenate([res.results[c]["out"] for c in range(N_CORES)], axis=0)
